# revision 12
# baseline (speedup 1.0000x reference)
"""Trainium2 Bass kernel for the circular drift-diffusion loss (batched expm).

Reference computes  loss = -mean_b log(relu(e_{idx_b}^T expm(t_b*A) p0_b) + eps)
with A a fixed 360x360 circular advection-diffusion operator, t_b in [0,1000),
p0_b a von Mises density, over a batch of 256.

Algorithm (per core; batch sharded 32/core over 8 cores):
  * Quantize t_b = m_b*T0 + r_b with T0 = 1000/2^K, m_b < 2^K.
  * M-chain: build M_j = expm(2^j*T0*A) by repeated squaring (prelude:
    ascending Taylor at T0, then K-2 squarings).  A squaring is 9 wide
    matmuls for S = M@M plus 9 PE transposes for S^T (the next stationary
    operand); the last squaring computes S^T directly (matmul with swapped
    operands) since its row form is never consumed.
  * Q-chain (decoupled, runs one level behind the M-chain off the critical
    path): p0 -> residual Taylor_DEG_R(r_b A) (Horner with host-precomputed
    r/k tables) -> per-level masked batched matvecs Q <- bit_j ? M_j Q : Q.
    The top TWO bits reuse M_{K-2}: bit K-2 one apply, bit K-1 two applies.
  * p0 built on device (minimax poly cos + Exp activation), selection via
    one-hot + PE column-sum, loss terms via exponent/mantissa split + Ln.

Fast path (chosen when the chain is short, k_bits <= 10): all matrix/vector
tiles bf16 -> PE matmuls 1 cycle/row (vs 4 for f32), transposes 1 (vs 2),
DVE elementwise gets 2-byte fast modes, DMA halves.  PSUM stays f32; p0's
phase pipeline stays f32.  Numpy bit-simulation keeps loss rel-err
2e-4..9e-3 for k_bits <= 10 plans (gate 2e-2); deeper chains fall back to
f32.  "f32r" mode (f32 tiles, matmul operands viewed as float32r) is the
full-precision alternative at the same wide-matmul rate.

Latency structure: inputs arrive on two HWDGE queues (SP + ACT) in
consumption order; per squaring the three S-row evacuations are staggered
behind their PSUM sub-groups, MT row 0 is evacuated in [P,120] pieces so the
next level's first matmuls unblock early, and rows 1/2 evacuate batched.
"""

import math

import numpy as np

# ---------------- static problem constants (hardcoded per contract) ----------
N = 360            # color mesh size
P = 120            # partition chunk (N = 3*P)
NCH = 3            # chunks
B = 256            # total batch
NCORES = 8
BL = B // NCORES   # per-core batch
QW = NCH * BL      # Q-chain tile width (96)
T_MAX = 1000.0
KAPPA = 400.0      # 1/SIGMA_INIT^2
EPS = 1e-5
TWO_PI = 6.283185307179586
# ln(1/(2*pi*i0e(400)))  [i0e(400) = 0.019953356281939987]
LNC = 2.076480848703078
# cos(sqrt(u)) on u in [0, pi^2] (|delta| folded to [0,pi]), minimax power
# basis c0..c5 (max err 1.75e-6 -> 7e-4 on log p0; tolerance is 2e-2)
COS_COEF = [0.9999982503105576, -0.4999925129381312, 0.0416590259231213,
            -0.0013857591185452258, 2.419643469550081e-05,
            -2.1969780329048054e-07]
# degree-8 Taylor-fit coefficients for the f32 fallback path (err 4e-14)
COS_COEF8 = [1.00000000e+00, -5.00000000e-01, 4.16666666e-02, -1.38888885e-03,
             2.48015646e-05, -2.75566515e-07, 2.08651966e-09, -1.13535474e-11,
             4.13131734e-14]

_COMPILED = {}

# fast-path dtype mode: "bf16" (2-byte tiles, PE 1cyc/row, DVE 2x modes) or
# "f32r" (f32 tiles, matmul operands viewed as float32r: PE 1cyc/row on wide
# matmuls at full f32 storage precision).  Chosen empirically on hardware.
FAST_MODE = "bf16"


def _taylor_deg(x, tol, lo):
    """Smallest d with x^(d+1)/(d+1)! < tol."""
    d = lo
    term = x ** (d + 1) / math.factorial(d + 1)
    while term > tol and d < 40:
        d += 1
        term *= x / (d + 1)
    return d


def _plan(anorm):
    """Choose (k_bits, deg_p, deg_r, mode) from ||A||_inf.  The time grid is
    T0 = T_MAX/2^k_bits; every squaring level applies one bit of the
    quantized delay."""
    xa = T_MAX * float(anorm)
    if xa <= 0.0:
        return 3, 4, 3, FAST_MODE

    def pick(c_lvl, c_pre, c_tay, tol_r, tol_p_num):
        k0 = max(3, min(16, math.ceil(math.log2(max(xa / 0.9, 2.0)))))
        best = None
        for k in range(max(3, k0 - 1), min(16, k0 + 2) + 1):
            x0 = xa / (1 << k)
            tol_p = min(max(tol_p_num / 2 ** (k / 2), 5e-8), 2e-5)
            dp = _taylor_deg(x0, tol_p, 4)
            dr = _taylor_deg(x0, tol_r, 3)
            cost = (k - 1) * c_lvl + (dp - 1) * c_pre + dr * c_tay
            if best is None or cost < best[0]:
                best = (cost, k, dp, dr)
        return best[1], best[2], best[3]

    fast_c = {"bf16": (2.2, 1.6, 0.6), "f32r": (2.5, 1.7, 0.7)}[FAST_MODE]
    k, dp, dr = pick(*fast_c, 2e-4, 3e-4)
    if k <= 10:
        return k, dp, dr, FAST_MODE
    k, dp, dr = pick(7.6, 4.5, 0.7, 1e-6, 3e-5)
    return k, dp, dr, "f32"


def _build_bass(k_bits, deg_p, deg_r, mode):
    """Construct the Bass program (SPMD; identical on all 8 cores)."""
    import concourse.tile as tile
    from concourse import bacc, mybir

    F32 = mybir.dt.float32
    R32 = mybir.dt.float32r
    BF = mybir.dt.bfloat16 if mode == "bf16" else F32
    MDT = BF if mode == "bf16" else mybir.dt.uint8   # mask dtype
    AF = mybir.ActivationFunctionType
    OP = mybir.AluOpType
    cos_coef = COS_COEF if mode != "f32" else COS_COEF8

    def mv(ap):
        # matmul operand view: f32r reinterpretation in f32r mode
        return ap.bitcast(R32) if mode == "f32r" else ap

    nc = bacc.Bacc("TRN2", target_bir_lowering=False, debug=False)

    def din(name, shape, dt=F32):
        return nc.dram_tensor(name, shape, dt, kind="ExternalInput").ap()

    d_xx = din("xx", [P, 5 * N + 3 * P], BF)   # packed X / X^T pieces
    d_cmir = din("cmir", [P, NCH + BL])        # [c_mesh chunks | init rep]
    d_qtab = din("qtab", [P, (deg_r + 1) * QW], BF)  # [rdk | one-hot]
    d_msk = din("msk", [P, k_bits * QW], MDT)  # bit masks (0/1), x3 chunks
    d_out = nc.dram_tensor("terms", [1, BL], F32, kind="ExternalOutput").ap()

    with tile.TileContext(nc) as tc:
        with (
            tc.tile_pool(name="const", bufs=1) as cpool,
            tc.tile_pool(name="mats", bufs=3) as mpool,
            tc.tile_pool(name="qp", bufs=3) as qpool,
            tc.tile_pool(name="tp", bufs=4) as tpool,
            tc.tile_pool(name="psb", bufs=3, space="PSUM") as psb,
            tc.tile_pool(name="pst", bufs=3, space="PSUM") as pstp,
            tc.tile_pool(name="pss", bufs=2, space="PSUM") as pss,
        ):
            # ---- input DMAs: few, packed, in consumption order ------------
            # xx layout: [XT00 | XNr0 | XTr1 | XNr1 | XTr2 | XNr2 | XT0rest]
            # so each DMA lands exactly what the next prelude matmuls need.
            XXW = 5 * N + 3 * P
            cuts = [0, P + N, P + 3 * N, P + 5 * N, XXW]
            XXT = []
            for j, (a, b) in enumerate(zip(cuts[:-1], cuts[1:])):
                xx_j = cpool.tile([P, b - a], BF, tag=f"xx{j}")
                XXT.append(xx_j)
            CMIR = cpool.tile([P, NCH + BL], F32, tag="cmir")
            QTAB = cpool.tile([P, (deg_r + 1) * QW], BF, tag="qtab")
            MSK = cpool.tile([P, k_bits * QW], MDT, tag="msk")
            # alternate the two HWDGE queues so issue overlaps (each
            # dma_start costs ~650ns of its sequencer)
            engs = [nc.sync, nc.scalar, nc.sync, nc.scalar]
            for j, (a, b) in enumerate(zip(cuts[:-1], cuts[1:])):
                engs[j].dma_start(XXT[j][:], d_xx[:, a:b])
            nc.sync.dma_start(QTAB[:], d_qtab[:])
            nc.scalar.dma_start(CMIR[:], d_cmir[:])
            nc.sync.dma_start(MSK[:], d_msk[:])
            CM = CMIR[:, 0:NCH]
            IREP = CMIR[:, NCH:NCH + BL]
            RDK = QTAB[:, 0:deg_r * QW]
            OH = QTAB[:, deg_r * QW:(deg_r + 1) * QW]

            def _xx(off, w):
                for j, (a, b) in enumerate(zip(cuts[:-1], cuts[1:])):
                    if a <= off and off + w <= b:
                        return XXT[j][:, off - a:off - a + w]
                raise AssertionError("xx slice crosses DMA boundary")

            def xn_s(c):
                # XN row-chunk c (rhs of prelude matmuls), contiguous
                return _xx(P + 2 * c * N, N)

            def xt_s(c, i):
                # XT block (row-chunk c, piece i) for prelude/taylor lhsT
                if c == 0:
                    o = 0 if i == 0 else P + 5 * N + (i - 1) * P
                else:
                    o = P + (2 * c - 1) * N + i * P
                return _xx(o, P)

            ONES = cpool.tile([P, 1], BF, tag="ones")
            nc.vector.memset(ONES[:], 1.0)
            BEXP = cpool.tile([P, 1], F32, tag="bexp")
            nc.vector.memset(BEXP[:], LNC - KAPPA)
            BLN0 = cpool.tile([1, 1], F32, tag="bln0")
            nc.vector.memset(BLN0[:], 0.0)
            LDUM = cpool.tile([1, 1], F32, tag="ldum")
            # identities built on device (no DMA)
            EYE = cpool.tile([P, NCH * N], BF, tag="eye")
            nc.vector.memset(EYE[:], 1.0)
            nc.gpsimd.affine_select(
                EYE[:].rearrange("p (c n) -> p c n", c=NCH),
                EYE[:].rearrange("p (c n) -> p c n", c=NCH),
                pattern=[[-P, NCH], [1, N]], compare_op=OP.is_equal,
                fill=0.0, base=0, channel_multiplier=-1,
            )
            E120 = cpool.tile([P, P], BF, tag="e120")
            nc.vector.memset(E120[:], 1.0)
            nc.gpsimd.affine_select(
                E120[:], E120[:], pattern=[[1, P]], compare_op=OP.is_equal,
                fill=0.0, base=0, channel_multiplier=-1,
            )

            def mm_group(ps, lhsT_of, rhs_of, i):
                # lhsT_of(c, i) -> [P,P] slice; rhs_of(c) -> [P,w] slice
                for c in range(NCH):
                    nc.tensor.matmul(
                        ps[:], lhsT=mv(lhsT_of(c, i)), rhs=mv(rhs_of(c)),
                        start=(c == 0), stop=(c == NCH - 1),
                    )

            def tile_b(tile_, c, i):
                return tile_[:, c * N + i * P: c * N + i * P + P]

            # ---- p0 (von Mises) in Q-layout [P, 3*BL] ---------------------
            Q0 = qpool.tile([P, QW], BF, tag="q")
            deg_c = len(cos_coef) - 1
            for c in range(NCH):
                qs = Q0[:, c * BL:(c + 1) * BL]
                dl = tpool.tile([P, BL], F32, tag="t0")
                # delta = init - c_mesh  (cos is even, sign irrelevant)
                nc.vector.tensor_scalar(dl[:], IREP[:], CM[:, c:c + 1], None,
                                        op0=OP.subtract)
                ab = tpool.tile([P, BL], F32, tag="t1")
                nc.scalar.activation(ab[:], dl[:], AF.Abs)
                fl = tpool.tile([P, BL], F32, tag="t2")
                nc.vector.tensor_scalar(fl[:], ab[:], -1.0, TWO_PI,
                                        op0=OP.mult, op1=OP.add)
                w = tpool.tile([P, BL], F32, tag="t3")
                nc.vector.tensor_tensor(w[:], ab[:], fl[:], op=OP.min)
                u = tpool.tile([P, BL], F32, tag="t0")
                nc.vector.tensor_tensor(u[:], w[:], w[:], op=OP.mult)
                h = tpool.tile([P, BL], F32, tag="t1")
                nc.vector.tensor_scalar(h[:], u[:], cos_coef[deg_c],
                                        cos_coef[deg_c - 1],
                                        op0=OP.mult, op1=OP.add)
                heng = nc.gpsimd if c == 1 else nc.vector
                for k in range(deg_c - 2, -1, -1):
                    heng.tensor_tensor(h[:], h[:], u[:], op=OP.mult)
                    heng.tensor_scalar(h[:], h[:], cos_coef[k], None,
                                       op0=OP.add)
                # p0 = exp(kappa*cos - kappa + lnC)
                nc.scalar.activation(qs, h[:], AF.Exp, bias=BEXP[:],
                                     scale=KAPPA)

            # ---- Q-chain step emitters (off the M-chain critical path) ----
            # residual Taylor: V <- Q0 + rdk_k*(X V), k=deg_r..1
            taylor_state = {"V": Q0, "k": deg_r}

            def taylor_step():
                k = taylor_state["k"]
                if k < 1:
                    return
                Vc = taylor_state["V"]
                ps = pss.tile([P, QW], F32, tag="ap")
                for i in range(NCH):
                    for c in range(NCH):
                        nc.tensor.matmul(
                            ps[:, i * BL:(i + 1) * BL],
                            lhsT=mv(xt_s(c, i)),
                            rhs=mv(Vc[:, c * BL:(c + 1) * BL]),
                            start=(c == 0), stop=(c == NCH - 1),
                        )
                Vn = qpool.tile([P, QW], BF, tag="v")
                nc.vector.tensor_tensor(Vn[:], ps[:],
                                        RDK[:, (k - 1) * QW: k * QW],
                                        op=OP.mult)
                nc.gpsimd.tensor_tensor(Vn[:], Vn[:], Q0[:], op=OP.add)
                taylor_state["V"] = Vn
                taylor_state["k"] = k - 1

            def apply_bit(MTj, q_tile, bit, blend_src=None):
                # Qn = bit ? M_j @ q : blend_src   (9 narrow mms + one blend)
                # the pass-through copy goes first: it only needs q, so it
                # overlaps the matmuls instead of serializing after them
                Qn = qpool.tile([P, QW], BF, tag="q")
                nc.gpsimd.tensor_copy(
                    Qn[:], (q_tile if blend_src is None else blend_src)[:])
                ps = pss.tile([P, QW], F32, tag="ap")
                for i in range(NCH):
                    for c in range(NCH):
                        nc.tensor.matmul(
                            ps[:, i * BL:(i + 1) * BL],
                            lhsT=mv(tile_b(MTj, c, i)),
                            rhs=mv(q_tile[:, c * BL:(c + 1) * BL]),
                            start=(c == 0), stop=(c == NCH - 1),
                        )
                nc.vector.copy_predicated(
                    Qn[:], MSK[:, bit * QW:(bit + 1) * QW], ps[:])
                return Qn

            # ---- prelude: ascending Taylor S = I + sum X^k/k! -------------
            S = mpool.tile([P, NCH * N], BF, tag="M")
            for c in range(NCH):
                nc.vector.tensor_tensor(S[:, c * N:(c + 1) * N], xn_s(c),
                                        EYE[:, c * N:(c + 1) * N], op=OP.add)
            T = None   # None -> XN accessor
            for k in range(2, deg_p + 1):
                Tn = mpool.tile([P, NCH * N], BF, tag="T")
                rhs_of = xn_s if T is None else (
                    lambda c, _T=T: _T[:, c * N:(c + 1) * N])
                for i in range(NCH):
                    ps = psb.tile([P, N], F32, tag="sq")
                    mm_group(ps, xt_s, rhs_of, i)
                    # scale-evac feeds the next step's matmuls; the S
                    # accumulation runs on DVE off the PE path
                    dst = Tn[:, i * N:(i + 1) * N]
                    if i == 1:
                        nc.gpsimd.tensor_scalar(dst, ps[:], 1.0 / k, None,
                                                op0=OP.mult)
                    else:
                        nc.scalar.mul(dst, ps[:], 1.0 / k)
                    nc.vector.tensor_tensor(S[:, i * N:(i + 1) * N],
                                            S[:, i * N:(i + 1) * N], dst,
                                            op=OP.add)
                T = Tn
                taylor_step()
            while taylor_state["k"] >= 1:   # drain (off critical path)
                taylor_step()
            QB = taylor_state["V"]

            ev = {"i": 0}

            def piece_evac(dst, src):
                e = ev["i"] % 3
                ev["i"] += 1
                if e == 0:
                    nc.vector.tensor_copy(dst, src)
                elif e == 1:
                    nc.scalar.copy(dst, src)
                else:
                    nc.gpsimd.tensor_copy(dst, src)

            def transpose_mq(MTt, Mt):
                # transposes ordered by source S-row (ib) so each trio only
                # waits its own row's evacuation; every [P,P] block
                # evacuates immediately so the next level's first matmuls
                # (which need MT row c piece 0 = S blocks (0,c)) unblock as
                # early as possible.
                for ib in range(NCH):
                    pt = pstp.tile([P, N], BF, tag="tr")
                    for cp in range(NCH):
                        nc.tensor.transpose(
                            mv(pt[:, cp * P:(cp + 1) * P]),
                            mv(Mt[:, ib * N + cp * P: ib * N + cp * P + P]),
                            mv(E120[:]),
                        )
                        piece_evac(MTt[:, cp * N + ib * P: cp * N + ib * P + P],
                                   pt[:, cp * P:(cp + 1) * P])

            # preload the Ln table set during the chain (ACT tables are
            # only otherwise touched at the very end; without this the set
            # switch lands on the loss critical path)
            nc.scalar.activation(LDUM[:], BLN0[:], AF.Ln, bias=BLN0[:],
                                 scale=1.0)

            MT = mpool.tile([P, NCH * N], BF, tag="MT")
            transpose_mq(MT, S)

            # ---- chain: squarings with lag-1 merged bit applies -----------
            # squaring s: M_s = M_{s-1}^2 (wide mms + transposes); bit s-1
            # applied right after the wide mms (its operands are a full level
            # old, so the PE never stalls on the Q-chain).
            n_sq = k_bits - 2
            for s in range(1, n_sq + 1):
                last = s == n_sq
                if not last:
                    Sn = mpool.tile([P, NCH * N], BF, tag="M")
                    for i in range(NCH):
                        ps = psb.tile([P, N], F32, tag="sq")
                        mm_group(ps, lambda c, ii, _M=MT: tile_b(_M, c, ii),
                                 lambda c, _S=S: _S[:, c * N:(c + 1) * N], i)
                        dst = Sn[:, i * N:(i + 1) * N]
                        if i == 0:
                            nc.gpsimd.tensor_copy(dst, ps[:])
                        elif i == 1:
                            nc.scalar.copy(dst, ps[:])
                        else:
                            # the last row gates this level's transposes:
                            # split it across DVE+ACT so it lands fastest
                            h = N // 2
                            nc.vector.tensor_copy(dst[:, 0:h], ps[:, 0:h])
                            nc.scalar.copy(dst[:, h:N], ps[:, h:N])
                else:
                    # last squaring: only M_{n_sq}^T is ever consumed (by the
                    # top-bit applies) -> compute S^T = M^T@M^T directly and
                    # skip the row form + transposes entirely
                    MTn = mpool.tile([P, NCH * N], BF, tag="MT")
                    for i in range(NCH):
                        ps = psb.tile([P, N], F32, tag="sq")
                        for c in range(NCH):
                            nc.tensor.matmul(
                                ps[:], lhsT=mv(tile_b(S, c, i)),
                                rhs=mv(MT[:, c * N: (c + 1) * N]),
                                start=(c == 0), stop=(c == NCH - 1),
                            )
                        eng = (nc.gpsimd, nc.scalar, nc.vector)[i]
                        if eng is nc.scalar:
                            nc.scalar.copy(MTn[:, i * N:(i + 1) * N], ps[:])
                        elif eng is nc.vector:
                            nc.vector.tensor_copy(MTn[:, i * N:(i + 1) * N],
                                                  ps[:])
                        else:
                            nc.gpsimd.tensor_copy(MTn[:, i * N:(i + 1) * N],
                                                  ps[:])
                # bit s-1 rides on M_{s-1} (= current MT) while evacs run
                QB = apply_bit(MT, QB, s - 1)
                if not last:
                    MTn = mpool.tile([P, NCH * N], BF, tag="MT")
                    transpose_mq(MTn, Sn)
                    S = Sn
                MT = MTn

            # ---- top two bits: single + double apply of M_{k-2} -----------
            QB = apply_bit(MT, QB, k_bits - 2)
            psy = pss.tile([P, QW], F32, tag="ap")
            for i in range(NCH):
                for c in range(NCH):
                    nc.tensor.matmul(
                        psy[:, i * BL:(i + 1) * BL],
                        lhsT=mv(tile_b(MT, c, i)),
                        rhs=mv(QB[:, c * BL:(c + 1) * BL]),
                        start=(c == 0), stop=(c == NCH - 1),
                    )
            Y1 = qpool.tile([P, QW], BF, tag="v")
            nc.vector.tensor_copy(Y1[:], psy[:])
            Vf = apply_bit(MT, Y1, k_bits - 1, blend_src=QB)

            # ---- selection + loss terms -----------------------------------
            sel = psb.tile([1, BL], F32, tag="sq")
            tmp = tpool.tile([P, QW], BF, tag="sel")
            nc.vector.tensor_tensor(tmp[:], Vf[:], OH[:], op=OP.mult)
            for c in range(NCH):
                nc.tensor.matmul(sel[:], lhsT=mv(ONES[:]),
                                 rhs=mv(tmp[:, c * BL:(c + 1) * BL]),
                                 start=(c == 0), stop=(c == NCH - 1))
            # ln(relu(psel)+eps) via exponent/mantissa split: the HW Ln
            # table degrades for huge args (psel can reach ~1e20 in the
            # weak-diffusion regime), so compute ln(m) + e*ln2 with m in
            # [1,2), which keeps the table in its accurate range.
            I32 = mybir.dt.int32
            rl = tpool.tile([1, BL], F32, tag="r0")
            nc.vector.tensor_scalar(rl[:], sel[:], 0.0, EPS,
                                    op0=OP.max, op1=OP.add)
            xi = rl[:].bitcast(I32)
            et = tpool.tile([1, BL], I32, tag="r2")
            nc.vector.tensor_scalar(et[:], xi, 23, None,
                                    op0=OP.arith_shift_right)
            ef = tpool.tile([1, BL], F32, tag="r3")
            nc.vector.tensor_copy(ef[:], et[:])
            mi = tpool.tile([1, BL], I32, tag="r4")
            nc.vector.tensor_scalar(mi[:], xi, 0x007FFFFF, 0x3F800000,
                                    op0=OP.bitwise_and, op1=OP.bitwise_or)
            lnm = tpool.tile([1, BL], F32, tag="r5")
            nc.scalar.activation(lnm[:], mi[:].bitcast(F32), AF.Ln,
                                 bias=BLN0[:], scale=1.0)
            terms = tpool.tile([1, BL], F32, tag="r1")
            # ef holds the biased exponent; fold the -127*ln2 into the mult
            nc.vector.tensor_scalar(terms[:], ef[:], 0.6931471805599453,
                                    -88.02969193111305,
                                    op0=OP.mult, op1=OP.add)
            nc.vector.tensor_tensor(terms[:], terms[:], lnm[:], op=OP.add)
            nc.sync.dma_start(d_out[:], terms[:])

    nc.compile()
    return nc


def _host_prep(c_mesh, gtheta, sigma_diff, init_color, delay_t, report_color):
    """Host-side glue: operator assembly (replicating reference f32 ops),
    plan selection, and per-core index/bit/layout arrays."""
    import ml_dtypes
    f32 = np.float32
    c = np.asarray(c_mesh, dtype=f32)
    g = np.asarray(gtheta, dtype=f32)
    s = np.asarray(sigma_diff, dtype=f32)[0]
    init = np.asarray(init_color, dtype=f32)
    t = np.asarray(delay_t, dtype=f32)
    rep = np.asarray(report_color, dtype=f32)

    d = (c[1] - c[0]).astype(f32)
    eye = np.eye(N, dtype=f32)
    up = np.roll(eye, -1, axis=1)
    dn = np.roll(eye, 1, axis=1)
    D1 = ((up - dn) / (f32(2.0) * d)).astype(f32)
    D2 = ((up - f32(2.0) * eye + dn) / (d * d)).astype(f32)
    A = ((s ** f32(2.0)) / f32(2.0) * D2 - D1 * g[None, :]).astype(f32)

    anorm = np.abs(A.astype(np.float64)).sum(axis=1).max()
    k_bits, deg_p, deg_r, mode = plan = _plan(anorm)
    bf = ml_dtypes.bfloat16 if mode == "bf16" else f32
    mdt = ml_dtypes.bfloat16 if mode == "bf16" else np.uint8
    T0 = T_MAX / (1 << k_bits)
    X = (A * f32(T0)).astype(f32)

    m = np.floor(t.astype(np.float64) / T0).astype(np.int64)
    m = np.clip(m, 0, (1 << k_bits) - 1)
    r = (t.astype(np.float64) - m * T0) / T0  # in X = T0*A units
    bits = ((m[:, None] >> np.arange(k_bits)[None, :]) & 1)     # [B, K]
    idx = np.argmin(np.abs(c[None, :] - rep[:, None]), axis=1)

    # packed matrix buffer in DMA/consumption order:
    # [XT00 | XNr0 | XTr1 | XNr1 | XTr2 | XNr2 | XT0rest]
    XT_ = np.ascontiguousarray(X.T)
    xx = np.concatenate([
        XT_[0:P, 0:P],
        X[0:P, :], XT_[P:2 * P, :],
        X[P:2 * P, :], XT_[2 * P:3 * P, :],
        X[2 * P:3 * P, :], XT_[0:P, P:N],
    ], axis=1)
    cm = np.ascontiguousarray(c.reshape(NCH, P).T)
    shared = {"xx": xx.astype(bf)}
    in_maps = []
    for core in range(NCORES):
        sl = slice(core * BL, (core + 1) * BL)
        irep = np.broadcast_to(init[sl][None, :], (P, BL)).astype(f32)
        cmir = np.concatenate([cm, irep], axis=1).astype(f32)
        # bit j replicated x3 (one copy per Q chunk) at [j*QW:(j+1)*QW]
        msk = np.broadcast_to(
            np.tile(bits[sl].T.reshape(k_bits, 1, BL), (1, NCH, 1))
            .reshape(1, k_bits * QW), (P, k_bits * QW)).astype(mdt)
        rdk = np.empty((deg_r, BL), f32)
        for k in range(1, deg_r + 1):
            rdk[k - 1] = (r[sl] / k).astype(f32)
        rdk = np.tile(rdk.reshape(deg_r, 1, BL), (1, NCH, 1)).reshape(
            1, deg_r * QW)
        oh = np.zeros((NCH, P, BL), f32)
        for b, ix in enumerate(idx[sl]):
            oh[ix // P, ix % P, b] = 1.0
        oh = oh.transpose(1, 0, 2).reshape(P, QW)
        qtab = np.concatenate(
            [np.broadcast_to(rdk, (P, deg_r * QW)), oh], axis=1).astype(bf)
        in_maps.append(dict(shared, cmir=cmir, msk=np.ascontiguousarray(msk),
                            qtab=np.ascontiguousarray(qtab)))
    return plan, in_maps


def _get_nc(plan):
    if plan not in _COMPILED:
        _COMPILED[plan] = _build_bass(*plan)
    return _COMPILED[plan]


def kernel(**inputs):
    from concourse.bass_utils import run_bass_kernel_spmd

    plan, in_maps = _host_prep(
        inputs["c_mesh"], inputs["gtheta"], inputs["sigma_diff"],
        inputs["init_color"], inputs["delay_t"], inputs["report_color"],
    )
    nc = _get_nc(plan)
    res = run_bass_kernel_spmd(nc, in_maps, list(range(NCORES)))
    terms = np.concatenate(
        [np.asarray(res.results[k]["terms"]).reshape(-1) for k in range(NCORES)]
    )
    loss = -np.mean(terms.astype(np.float64))
    return np.asarray(loss, dtype=np.float32)


# revision 13
# speedup vs baseline: 1.0320x; 1.0320x over previous
"""Trainium2 Bass kernel for the circular drift-diffusion loss (batched expm).

Reference computes  loss = -mean_b log(relu(e_{idx_b}^T expm(t_b*A) p0_b) + eps)
with A a fixed 360x360 circular advection-diffusion operator, t_b in [0,1000),
p0_b a von Mises density, over a batch of 256.

Algorithm (per core; batch sharded 32/core over 8 cores):
  * Quantize t_b = m_b*T0 + r_b with T0 = 1000/2^K, m_b < 2^K.
  * M-chain: build M_j = expm(2^j*T0*A) by repeated squaring (prelude:
    ascending Taylor at T0, then K-2 squarings).  A squaring is 9 wide
    matmuls for S = M@M plus 9 PE transposes for S^T (the next stationary
    operand); the last squaring computes S^T directly (matmul with swapped
    operands) since its row form is never consumed.
  * Q-chain (decoupled, runs one level behind the M-chain off the critical
    path): p0 -> residual Taylor_DEG_R(r_b A) (Horner with host-precomputed
    r/k tables) -> per-level masked batched matvecs Q <- bit_j ? M_j Q : Q.
    The top TWO bits reuse M_{K-2}: bit K-2 one apply, bit K-1 two applies.
  * p0 built on device (minimax poly cos + Exp activation), selection via
    one-hot + PE column-sum, loss terms via exponent/mantissa split + Ln.

Fast path (chosen when the chain is short, k_bits <= 10): all matrix/vector
tiles bf16 -> PE matmuls 1 cycle/row (vs 4 for f32), transposes 1 (vs 2),
DVE elementwise gets 2-byte fast modes, DMA halves.  PSUM stays f32; p0's
phase pipeline stays f32.  Numpy bit-simulation keeps loss rel-err
2e-4..9e-3 for k_bits <= 10 plans (gate 2e-2); deeper chains fall back to
f32.  "f32r" mode (f32 tiles, matmul operands viewed as float32r) is the
full-precision alternative at the same wide-matmul rate.

Latency structure: inputs arrive on two HWDGE queues (SP + ACT) in
consumption order; per squaring the three S-row evacuations are staggered
behind their PSUM sub-groups, MT row 0 is evacuated in [P,120] pieces so the
next level's first matmuls unblock early, and rows 1/2 evacuate batched.
"""

import math

import numpy as np

# ---------------- static problem constants (hardcoded per contract) ----------
N = 360            # color mesh size
P = 120            # partition chunk (N = 3*P)
NCH = 3            # chunks
B = 256            # total batch
NCORES = 8
BL = B // NCORES   # per-core batch
QW = NCH * BL      # Q-chain tile width (96)
T_MAX = 1000.0
KAPPA = 400.0      # 1/SIGMA_INIT^2
EPS = 1e-5
TWO_PI = 6.283185307179586
# ln(1/(2*pi*i0e(400)))  [i0e(400) = 0.019953356281939987]
LNC = 2.076480848703078
# cos(sqrt(u)) on u in [0, pi^2] (|delta| folded to [0,pi]), minimax power
# basis c0..c5 (max err 1.75e-6 -> 7e-4 on log p0; tolerance is 2e-2)
COS_COEF = [0.9999982503105576, -0.4999925129381312, 0.0416590259231213,
            -0.0013857591185452258, 2.419643469550081e-05,
            -2.1969780329048054e-07]
# degree-8 Taylor-fit coefficients for the f32 fallback path (err 4e-14)
COS_COEF8 = [1.00000000e+00, -5.00000000e-01, 4.16666666e-02, -1.38888885e-03,
             2.48015646e-05, -2.75566515e-07, 2.08651966e-09, -1.13535474e-11,
             4.13131734e-14]

_COMPILED = {}

# fast-path dtype mode: "bf16" (2-byte tiles, PE 1cyc/row, DVE 2x modes) or
# "f32r" (f32 tiles, matmul operands viewed as float32r: PE 1cyc/row on wide
# matmuls at full f32 storage precision).  Chosen empirically on hardware.
FAST_MODE = "bf16"


def _taylor_deg(x, tol, lo):
    """Smallest d with x^(d+1)/(d+1)! < tol."""
    d = lo
    term = x ** (d + 1) / math.factorial(d + 1)
    while term > tol and d < 40:
        d += 1
        term *= x / (d + 1)
    return d


def _plan(anorm):
    """Choose (k_bits, deg_p, deg_r, mode) from ||A||_inf.  The time grid is
    T0 = T_MAX/2^k_bits; every squaring level applies one bit of the
    quantized delay."""
    xa = T_MAX * float(anorm)
    if xa <= 0.0:
        return 3, 4, 3, FAST_MODE

    def pick(c_lvl, c_pre, c_tay, tol_r, tol_p_num):
        k0 = max(3, min(16, math.ceil(math.log2(max(xa / 0.9, 2.0)))))
        best = None
        for k in range(max(3, k0 - 1), min(16, k0 + 2) + 1):
            x0 = xa / (1 << k)
            tol_p = min(max(tol_p_num / 2 ** (k / 2), 5e-8), 2e-5)
            dp = _taylor_deg(x0, tol_p, 4)
            dr = _taylor_deg(x0, tol_r, 3)
            cost = (k - 1) * c_lvl + (dp - 1) * c_pre + dr * c_tay
            if best is None or cost < best[0]:
                best = (cost, k, dp, dr)
        return best[1], best[2], best[3]

    fast_c = {"bf16": (2.2, 1.6, 0.6), "f32r": (2.5, 1.7, 0.7)}[FAST_MODE]
    k, dp, dr = pick(*fast_c, 2e-4, 3e-4)
    if k <= 10:
        return k, dp, dr, FAST_MODE
    k, dp, dr = pick(7.6, 4.5, 0.7, 1e-6, 3e-5)
    return k, dp, dr, "f32"


def _build_bass(k_bits, deg_p, deg_r, mode):
    """Construct the Bass program (SPMD; identical on all 8 cores)."""
    import concourse.tile as tile
    from concourse import bacc, mybir

    F32 = mybir.dt.float32
    R32 = mybir.dt.float32r
    BF = mybir.dt.bfloat16 if mode == "bf16" else F32
    MDT = BF if mode == "bf16" else mybir.dt.uint8   # mask dtype
    AF = mybir.ActivationFunctionType
    OP = mybir.AluOpType
    cos_coef = COS_COEF if mode != "f32" else COS_COEF8

    def mv(ap):
        # matmul operand view: f32r reinterpretation in f32r mode
        return ap.bitcast(R32) if mode == "f32r" else ap

    nc = bacc.Bacc("TRN2", target_bir_lowering=False, debug=False)

    def din(name, shape, dt=F32):
        return nc.dram_tensor(name, shape, dt, kind="ExternalInput").ap()

    d_xx = din("xx", [P, 5 * N + 3 * P], BF)   # packed X / X^T pieces
    d_cmir = din("cmir", [P, NCH + BL])        # [c_mesh chunks | init rep]
    d_qtab = din("qtab", [P, (deg_r + 1) * QW], BF)  # [rdk | one-hot]
    d_msk = din("msk", [P, k_bits * QW], MDT)  # bit masks (0/1), x3 chunks
    d_out = nc.dram_tensor("terms", [1, BL], F32, kind="ExternalOutput").ap()

    with tile.TileContext(nc) as tc:
        with (
            tc.tile_pool(name="const", bufs=1) as cpool,
            tc.tile_pool(name="mats", bufs=3) as mpool,
            tc.tile_pool(name="qp", bufs=3) as qpool,
            tc.tile_pool(name="tp", bufs=4) as tpool,
            tc.tile_pool(name="psb", bufs=3, space="PSUM") as psb,
            tc.tile_pool(name="pst", bufs=3, space="PSUM") as pstp,
            tc.tile_pool(name="pss", bufs=2, space="PSUM") as pss,
        ):
            # ---- input DMAs: few, packed, in consumption order ------------
            # xx layout: [XT00 | XNr0 | XTr1 | XNr1 | XTr2 | XNr2 | XT0rest]
            # so each DMA lands exactly what the next prelude matmuls need.
            XXW = 5 * N + 3 * P
            cuts = [0, P + N, P + 3 * N, P + 5 * N, XXW]
            XXT = []
            for j, (a, b) in enumerate(zip(cuts[:-1], cuts[1:])):
                xx_j = cpool.tile([P, b - a], BF, tag=f"xx{j}")
                XXT.append(xx_j)
            CMIR = cpool.tile([P, NCH + BL], F32, tag="cmir")
            QTAB = cpool.tile([P, (deg_r + 1) * QW], BF, tag="qtab")
            MSK = cpool.tile([P, k_bits * QW], MDT, tag="msk")
            # alternate the two HWDGE queues so issue overlaps (each
            # dma_start costs ~650ns of its sequencer)
            engs = [nc.sync, nc.scalar, nc.sync, nc.scalar]
            for j, (a, b) in enumerate(zip(cuts[:-1], cuts[1:])):
                engs[j].dma_start(XXT[j][:], d_xx[:, a:b])
            nc.sync.dma_start(QTAB[:], d_qtab[:])
            nc.scalar.dma_start(CMIR[:], d_cmir[:])
            nc.sync.dma_start(MSK[:], d_msk[:])
            CM = CMIR[:, 0:NCH]
            IREP = CMIR[:, NCH:NCH + BL]
            RDK = QTAB[:, 0:deg_r * QW]
            OH = QTAB[:, deg_r * QW:(deg_r + 1) * QW]

            def _xx(off, w):
                for j, (a, b) in enumerate(zip(cuts[:-1], cuts[1:])):
                    if a <= off and off + w <= b:
                        return XXT[j][:, off - a:off - a + w]
                raise AssertionError("xx slice crosses DMA boundary")

            def xn_s(c):
                # XN row-chunk c (rhs of prelude matmuls), contiguous
                return _xx(P + 2 * c * N, N)

            def xt_s(c, i):
                # XT block (row-chunk c, piece i) for prelude/taylor lhsT
                if c == 0:
                    o = 0 if i == 0 else P + 5 * N + (i - 1) * P
                else:
                    o = P + (2 * c - 1) * N + i * P
                return _xx(o, P)

            ONES = cpool.tile([P, 1], BF, tag="ones")
            nc.vector.memset(ONES[:], 1.0)
            BEXP = cpool.tile([P, 1], F32, tag="bexp")
            nc.vector.memset(BEXP[:], LNC - KAPPA)
            BLN0 = cpool.tile([1, 1], F32, tag="bln0")
            nc.vector.memset(BLN0[:], 0.0)
            LDUM = cpool.tile([1, 1], F32, tag="ldum")
            # identities built on device (no DMA)
            EYE = cpool.tile([P, NCH * N], BF, tag="eye")
            nc.vector.memset(EYE[:], 1.0)
            nc.gpsimd.affine_select(
                EYE[:].rearrange("p (c n) -> p c n", c=NCH),
                EYE[:].rearrange("p (c n) -> p c n", c=NCH),
                pattern=[[-P, NCH], [1, N]], compare_op=OP.is_equal,
                fill=0.0, base=0, channel_multiplier=-1,
            )
            E120 = cpool.tile([P, P], BF, tag="e120")
            nc.vector.memset(E120[:], 1.0)
            nc.gpsimd.affine_select(
                E120[:], E120[:], pattern=[[1, P]], compare_op=OP.is_equal,
                fill=0.0, base=0, channel_multiplier=-1,
            )

            def mm_group(ps, lhsT_of, rhs_of, i):
                # lhsT_of(c, i) -> [P,P] slice; rhs_of(c) -> [P,w] slice
                for c in range(NCH):
                    nc.tensor.matmul(
                        ps[:], lhsT=mv(lhsT_of(c, i)), rhs=mv(rhs_of(c)),
                        start=(c == 0), stop=(c == NCH - 1),
                    )

            def tile_b(tile_, c, i):
                return tile_[:, c * N + i * P: c * N + i * P + P]

            # ---- p0 (von Mises) in Q-layout [P, 3*BL] ---------------------
            Q0 = qpool.tile([P, QW], BF, tag="q")
            deg_c = len(cos_coef) - 1
            for c in range(NCH):
                qs = Q0[:, c * BL:(c + 1) * BL]
                dl = tpool.tile([P, BL], F32, tag="t0")
                # delta = init - c_mesh  (cos is even, sign irrelevant)
                nc.vector.tensor_scalar(dl[:], IREP[:], CM[:, c:c + 1], None,
                                        op0=OP.subtract)
                ab = tpool.tile([P, BL], F32, tag="t1")
                nc.scalar.activation(ab[:], dl[:], AF.Abs)
                fl = tpool.tile([P, BL], F32, tag="t2")
                nc.vector.tensor_scalar(fl[:], ab[:], -1.0, TWO_PI,
                                        op0=OP.mult, op1=OP.add)
                w = tpool.tile([P, BL], F32, tag="t3")
                nc.vector.tensor_tensor(w[:], ab[:], fl[:], op=OP.min)
                u = tpool.tile([P, BL], F32, tag="t0")
                nc.vector.tensor_tensor(u[:], w[:], w[:], op=OP.mult)
                h = tpool.tile([P, BL], F32, tag="t1")
                nc.vector.tensor_scalar(h[:], u[:], cos_coef[deg_c],
                                        cos_coef[deg_c - 1],
                                        op0=OP.mult, op1=OP.add)
                heng = nc.gpsimd if c == 1 else nc.vector
                for k in range(deg_c - 2, -1, -1):
                    heng.tensor_tensor(h[:], h[:], u[:], op=OP.mult)
                    heng.tensor_scalar(h[:], h[:], cos_coef[k], None,
                                       op0=OP.add)
                # p0 = exp(kappa*cos - kappa + lnC)
                nc.scalar.activation(qs, h[:], AF.Exp, bias=BEXP[:],
                                     scale=KAPPA)

            # ---- Q-chain step emitters (off the M-chain critical path) ----
            # residual Taylor: V <- Q0 + rdk_k*(X V), k=deg_r..1
            taylor_state = {"V": Q0, "k": deg_r}

            def taylor_step():
                k = taylor_state["k"]
                if k < 1:
                    return
                Vc = taylor_state["V"]
                ps = pss.tile([P, QW], F32, tag="ap")
                for i in range(NCH):
                    for c in range(NCH):
                        nc.tensor.matmul(
                            ps[:, i * BL:(i + 1) * BL],
                            lhsT=mv(xt_s(c, i)),
                            rhs=mv(Vc[:, c * BL:(c + 1) * BL]),
                            start=(c == 0), stop=(c == NCH - 1),
                        )
                Vn = qpool.tile([P, QW], BF, tag="v")
                nc.vector.tensor_tensor(Vn[:], ps[:],
                                        RDK[:, (k - 1) * QW: k * QW],
                                        op=OP.mult)
                nc.gpsimd.tensor_tensor(Vn[:], Vn[:], Q0[:], op=OP.add)
                taylor_state["V"] = Vn
                taylor_state["k"] = k - 1

            def apply_bit(MTj, q_tile, bit, blend_src=None):
                # Qn = bit ? M_j @ q : blend_src   (9 narrow mms + one blend)
                # the pass-through copy goes first: it only needs q, so it
                # overlaps the matmuls instead of serializing after them
                Qn = qpool.tile([P, QW], BF, tag="q")
                nc.gpsimd.tensor_copy(
                    Qn[:], (q_tile if blend_src is None else blend_src)[:])
                ps = pss.tile([P, QW], F32, tag="ap")
                for i in range(NCH):
                    for c in range(NCH):
                        nc.tensor.matmul(
                            ps[:, i * BL:(i + 1) * BL],
                            lhsT=mv(tile_b(MTj, c, i)),
                            rhs=mv(q_tile[:, c * BL:(c + 1) * BL]),
                            start=(c == 0), stop=(c == NCH - 1),
                        )
                nc.vector.copy_predicated(
                    Qn[:], MSK[:, bit * QW:(bit + 1) * QW], ps[:])
                return Qn

            # ---- prelude: ascending Taylor S = I + sum X^k/k! -------------
            S = mpool.tile([P, NCH * N], BF, tag="M")
            for c in range(NCH):
                nc.vector.tensor_tensor(S[:, c * N:(c + 1) * N], xn_s(c),
                                        EYE[:, c * N:(c + 1) * N], op=OP.add)
            T = None   # None -> XN accessor
            for k in range(2, deg_p + 1):
                Tn = mpool.tile([P, NCH * N], BF, tag="T")
                rhs_of = xn_s if T is None else (
                    lambda c, _T=T: _T[:, c * N:(c + 1) * N])
                for i in range(NCH):
                    ps = psb.tile([P, N], F32, tag="sq")
                    mm_group(ps, xt_s, rhs_of, i)
                    # scale-evac feeds the next step's matmuls; the S
                    # accumulation runs on DVE off the PE path
                    dst = Tn[:, i * N:(i + 1) * N]
                    if i == 1:
                        nc.gpsimd.tensor_scalar(dst, ps[:], 1.0 / k, None,
                                                op0=OP.mult)
                    else:
                        nc.scalar.mul(dst, ps[:], 1.0 / k)
                    nc.vector.tensor_tensor(S[:, i * N:(i + 1) * N],
                                            S[:, i * N:(i + 1) * N], dst,
                                            op=OP.add)
                T = Tn
                taylor_step()
            while taylor_state["k"] >= 1:   # drain (off critical path)
                taylor_step()
            QB = taylor_state["V"]

            def transpose_trio(MTt, Mt, ib, eng):
                # transpose S row-chunk ib's three [P,P] blocks into one PSUM
                # bank, then ONE strided evacuation writing piece ib of all
                # three MT rows -- exactly the lhsT set the next level's
                # matmul group i=ib consumes.  (One read per PSUM bank: the
                # dependency tracker is bank-granular, so interleaving reads
                # between the transposes would serialize them.)
                pt = pstp.tile([P, N], BF, tag="tr")
                for cp in range(NCH):
                    nc.tensor.transpose(
                        mv(pt[:, cp * P:(cp + 1) * P]),
                        mv(Mt[:, ib * N + cp * P: ib * N + cp * P + P]),
                        mv(E120[:]),
                    )
                dst = MTt[:].rearrange("p (c n) -> p c n", c=NCH)[
                    :, :, ib * P:(ib + 1) * P]
                srcv = pt[:].rearrange("p (c q) -> p c q", c=NCH)
                if eng is nc.vector:
                    nc.vector.tensor_copy(dst, srcv)
                elif eng is nc.scalar:
                    nc.scalar.copy(dst, srcv)
                else:
                    nc.gpsimd.tensor_copy(dst, srcv)

            def transpose_mq(MTt, Mt):
                for ib in range(NCH):
                    transpose_trio(MTt, Mt, ib,
                                   (nc.vector, nc.scalar, nc.gpsimd)[ib])

            # preload the Ln table set mid-kernel: reading the last p0
            # exp's output pins this AFTER the exps in the ACT schedule, so
            # the natural_log set switch lands in chain idle time instead of
            # on the loss critical path
            nc.scalar.activation(LDUM[:], Q0[0:1, QW - 1:QW], AF.Ln,
                                 bias=BLN0[:], scale=1.0)

            MT = mpool.tile([P, NCH * N], BF, tag="MT")
            transpose_mq(MT, S)

            # ---- chain: squarings with lag-1 merged bit applies -----------
            # squaring s: M_s = M_{s-1}^2 (wide mms + transposes); bit s-1
            # applied right after the wide mms (its operands are a full level
            # old, so the PE never stalls on the Q-chain).
            n_sq = k_bits - 2
            for s in range(1, n_sq + 1):
                last = s == n_sq
                if not last:
                    Sn = mpool.tile([P, NCH * N], BF, tag="M")
                    for i in range(NCH):
                        ps = psb.tile([P, N], F32, tag="sq")
                        mm_group(ps, lambda c, ii, _M=MT: tile_b(_M, c, ii),
                                 lambda c, _S=S: _S[:, c * N:(c + 1) * N], i)
                        dst = Sn[:, i * N:(i + 1) * N]
                        if i == 0:
                            nc.gpsimd.tensor_copy(dst, ps[:])
                        elif i == 1:
                            nc.scalar.copy(dst, ps[:])
                        else:
                            # the last row gates this level's transposes:
                            # split it across DVE+ACT so it lands fastest
                            h = N // 2
                            nc.vector.tensor_copy(dst[:, 0:h], ps[:, 0:h])
                            nc.scalar.copy(dst[:, h:N], ps[:, h:N])
                else:
                    # last squaring: only M_{n_sq}^T is ever consumed (by the
                    # top-bit applies) -> compute S^T = M^T@M^T directly and
                    # skip the row form + transposes entirely
                    MTn = mpool.tile([P, NCH * N], BF, tag="MT")
                    for i in range(NCH):
                        ps = psb.tile([P, N], F32, tag="sq")
                        for c in range(NCH):
                            nc.tensor.matmul(
                                ps[:], lhsT=mv(tile_b(S, c, i)),
                                rhs=mv(MT[:, c * N: (c + 1) * N]),
                                start=(c == 0), stop=(c == NCH - 1),
                            )
                        eng = (nc.gpsimd, nc.scalar, nc.vector)[i]
                        if eng is nc.scalar:
                            nc.scalar.copy(MTn[:, i * N:(i + 1) * N], ps[:])
                        elif eng is nc.vector:
                            nc.vector.tensor_copy(MTn[:, i * N:(i + 1) * N],
                                                  ps[:])
                        else:
                            nc.gpsimd.tensor_copy(MTn[:, i * N:(i + 1) * N],
                                                  ps[:])
                if not last:
                    MTn = mpool.tile([P, NCH * N], BF, tag="MT")
                    transpose_trio(MTn, Sn, 0, nc.vector)
                    transpose_trio(MTn, Sn, 1, nc.scalar)
                    # bit s-1 rides on M_{s-1} while row 2 evacuates
                    QB = apply_bit(MT, QB, s - 1)
                    transpose_trio(MTn, Sn, 2, nc.gpsimd)
                    S = Sn
                else:
                    QB = apply_bit(MT, QB, s - 1)
                MT = MTn

            # ---- top two bits: single + double apply of M_{k-2} -----------
            QB = apply_bit(MT, QB, k_bits - 2)
            psy = pss.tile([P, QW], F32, tag="ap")
            for i in range(NCH):
                for c in range(NCH):
                    nc.tensor.matmul(
                        psy[:, i * BL:(i + 1) * BL],
                        lhsT=mv(tile_b(MT, c, i)),
                        rhs=mv(QB[:, c * BL:(c + 1) * BL]),
                        start=(c == 0), stop=(c == NCH - 1),
                    )
            Y1 = qpool.tile([P, QW], BF, tag="v")
            nc.vector.tensor_copy(Y1[:], psy[:])
            Vf = apply_bit(MT, Y1, k_bits - 1, blend_src=QB)

            # ---- selection + loss terms -----------------------------------
            sel = psb.tile([1, BL], F32, tag="sq")
            tmp = tpool.tile([P, QW], BF, tag="sel")
            nc.vector.tensor_tensor(tmp[:], Vf[:], OH[:], op=OP.mult)
            for c in range(NCH):
                nc.tensor.matmul(sel[:], lhsT=mv(ONES[:]),
                                 rhs=mv(tmp[:, c * BL:(c + 1) * BL]),
                                 start=(c == 0), stop=(c == NCH - 1))
            # ln(relu(psel)+eps) via exponent/mantissa split: the HW Ln
            # table degrades for huge args (psel can reach ~1e20 in the
            # weak-diffusion regime), so compute ln(m) + e*ln2 with m in
            # [1,2), which keeps the table in its accurate range.
            I32 = mybir.dt.int32
            rl = tpool.tile([1, BL], F32, tag="r0")
            nc.vector.tensor_scalar(rl[:], sel[:], 0.0, EPS,
                                    op0=OP.max, op1=OP.add)
            xi = rl[:].bitcast(I32)
            et = tpool.tile([1, BL], I32, tag="r2")
            nc.vector.tensor_scalar(et[:], xi, 23, None,
                                    op0=OP.arith_shift_right)
            ef = tpool.tile([1, BL], F32, tag="r3")
            nc.vector.tensor_copy(ef[:], et[:])
            mi = tpool.tile([1, BL], I32, tag="r4")
            nc.vector.tensor_scalar(mi[:], xi, 0x007FFFFF, 0x3F800000,
                                    op0=OP.bitwise_and, op1=OP.bitwise_or)
            lnm = tpool.tile([1, BL], F32, tag="r5")
            nc.scalar.activation(lnm[:], mi[:].bitcast(F32), AF.Ln,
                                 bias=BLN0[:], scale=1.0)
            terms = tpool.tile([1, BL], F32, tag="r1")
            # ef holds the biased exponent; fold the -127*ln2 into the mult
            nc.vector.tensor_scalar(terms[:], ef[:], 0.6931471805599453,
                                    -88.02969193111305,
                                    op0=OP.mult, op1=OP.add)
            nc.vector.tensor_tensor(terms[:], terms[:], lnm[:], op=OP.add)
            nc.sync.dma_start(d_out[:], terms[:])

    nc.compile()
    return nc


def _host_prep(c_mesh, gtheta, sigma_diff, init_color, delay_t, report_color):
    """Host-side glue: operator assembly (replicating reference f32 ops),
    plan selection, and per-core index/bit/layout arrays."""
    import ml_dtypes
    f32 = np.float32
    c = np.asarray(c_mesh, dtype=f32)
    g = np.asarray(gtheta, dtype=f32)
    s = np.asarray(sigma_diff, dtype=f32)[0]
    init = np.asarray(init_color, dtype=f32)
    t = np.asarray(delay_t, dtype=f32)
    rep = np.asarray(report_color, dtype=f32)

    d = (c[1] - c[0]).astype(f32)
    eye = np.eye(N, dtype=f32)
    up = np.roll(eye, -1, axis=1)
    dn = np.roll(eye, 1, axis=1)
    D1 = ((up - dn) / (f32(2.0) * d)).astype(f32)
    D2 = ((up - f32(2.0) * eye + dn) / (d * d)).astype(f32)
    A = ((s ** f32(2.0)) / f32(2.0) * D2 - D1 * g[None, :]).astype(f32)

    anorm = np.abs(A.astype(np.float64)).sum(axis=1).max()
    k_bits, deg_p, deg_r, mode = plan = _plan(anorm)
    bf = ml_dtypes.bfloat16 if mode == "bf16" else f32
    mdt = ml_dtypes.bfloat16 if mode == "bf16" else np.uint8
    T0 = T_MAX / (1 << k_bits)
    X = (A * f32(T0)).astype(f32)

    m = np.floor(t.astype(np.float64) / T0).astype(np.int64)
    m = np.clip(m, 0, (1 << k_bits) - 1)
    r = (t.astype(np.float64) - m * T0) / T0  # in X = T0*A units
    bits = ((m[:, None] >> np.arange(k_bits)[None, :]) & 1)     # [B, K]
    idx = np.argmin(np.abs(c[None, :] - rep[:, None]), axis=1)

    # packed matrix buffer in DMA/consumption order:
    # [XT00 | XNr0 | XTr1 | XNr1 | XTr2 | XNr2 | XT0rest]
    XT_ = np.ascontiguousarray(X.T)
    xx = np.concatenate([
        XT_[0:P, 0:P],
        X[0:P, :], XT_[P:2 * P, :],
        X[P:2 * P, :], XT_[2 * P:3 * P, :],
        X[2 * P:3 * P, :], XT_[0:P, P:N],
    ], axis=1)
    cm = np.ascontiguousarray(c.reshape(NCH, P).T)
    shared = {"xx": xx.astype(bf)}
    in_maps = []
    for core in range(NCORES):
        sl = slice(core * BL, (core + 1) * BL)
        irep = np.broadcast_to(init[sl][None, :], (P, BL)).astype(f32)
        cmir = np.concatenate([cm, irep], axis=1).astype(f32)
        # bit j replicated x3 (one copy per Q chunk) at [j*QW:(j+1)*QW]
        msk = np.broadcast_to(
            np.tile(bits[sl].T.reshape(k_bits, 1, BL), (1, NCH, 1))
            .reshape(1, k_bits * QW), (P, k_bits * QW)).astype(mdt)
        rdk = np.empty((deg_r, BL), f32)
        for k in range(1, deg_r + 1):
            rdk[k - 1] = (r[sl] / k).astype(f32)
        rdk = np.tile(rdk.reshape(deg_r, 1, BL), (1, NCH, 1)).reshape(
            1, deg_r * QW)
        oh = np.zeros((NCH, P, BL), f32)
        for b, ix in enumerate(idx[sl]):
            oh[ix // P, ix % P, b] = 1.0
        oh = oh.transpose(1, 0, 2).reshape(P, QW)
        qtab = np.concatenate(
            [np.broadcast_to(rdk, (P, deg_r * QW)), oh], axis=1).astype(bf)
        in_maps.append(dict(shared, cmir=cmir, msk=np.ascontiguousarray(msk),
                            qtab=np.ascontiguousarray(qtab)))
    return plan, in_maps


def _get_nc(plan):
    if plan not in _COMPILED:
        _COMPILED[plan] = _build_bass(*plan)
    return _COMPILED[plan]


def kernel(**inputs):
    from concourse.bass_utils import run_bass_kernel_spmd

    plan, in_maps = _host_prep(
        inputs["c_mesh"], inputs["gtheta"], inputs["sigma_diff"],
        inputs["init_color"], inputs["delay_t"], inputs["report_color"],
    )
    nc = _get_nc(plan)
    res = run_bass_kernel_spmd(nc, in_maps, list(range(NCORES)))
    terms = np.concatenate(
        [np.asarray(res.results[k]["terms"]).reshape(-1) for k in range(NCORES)]
    )
    loss = -np.mean(terms.astype(np.float64))
    return np.asarray(loss, dtype=np.float32)


# revision 14
# speedup vs baseline: 1.0943x; 1.0604x over previous
"""Trainium2 Bass kernel for the circular drift-diffusion loss (batched expm).

Reference computes  loss = -mean_b log(relu(e_{idx_b}^T expm(t_b*A) p0_b) + eps)
with A a fixed 360x360 circular advection-diffusion operator, t_b in [0,1000),
p0_b a von Mises density, over a batch of 256.

Algorithm (per core; batch sharded 32/core over 8 cores):
  * Quantize t_b = m_b*T0 + r_b with T0 = 1000/2^K, m_b < 2^K.
  * M-chain: build M_j = expm(2^j*T0*A) by repeated squaring (prelude:
    ascending Taylor at T0, then K-2 squarings).  A squaring is 9 wide
    matmuls for S = M@M plus 9 PE transposes for S^T (the next stationary
    operand); the last squaring computes S^T directly (matmul with swapped
    operands) since its row form is never consumed.
  * Q-chain (decoupled, runs one level behind the M-chain off the critical
    path): p0 -> residual Taylor_DEG_R(r_b A) (Horner with host-precomputed
    r/k tables) -> per-level masked batched matvecs Q <- bit_j ? M_j Q : Q.
    The top TWO bits reuse M_{K-2}: bit K-2 one apply, bit K-1 two applies.
  * p0 built on device (minimax poly cos + Exp activation), selection via
    one-hot + PE column-sum, loss terms via exponent/mantissa split + Ln.

Fast path (chosen when the chain is short, k_bits <= 10): all matrix/vector
tiles bf16 -> PE matmuls 1 cycle/row (vs 4 for f32), transposes 1 (vs 2),
DVE elementwise gets 2-byte fast modes, DMA halves.  PSUM stays f32; p0's
phase pipeline stays f32.  Numpy bit-simulation keeps loss rel-err
2e-4..9e-3 for k_bits <= 10 plans (gate 2e-2); deeper chains fall back to
f32.  "f32r" mode (f32 tiles, matmul operands viewed as float32r) is the
full-precision alternative at the same wide-matmul rate.

Latency structure: inputs arrive on two HWDGE queues (SP + ACT) in
consumption order; per squaring the three S-row evacuations are staggered
behind their PSUM sub-groups, MT row 0 is evacuated in [P,120] pieces so the
next level's first matmuls unblock early, and rows 1/2 evacuate batched.
"""

import math

import numpy as np

# ---------------- static problem constants (hardcoded per contract) ----------
N = 360            # color mesh size
P = 120            # partition chunk (N = 3*P)
NCH = 3            # chunks
B = 256            # total batch
NCORES = 8
BL = B // NCORES   # per-core batch
QW = NCH * BL      # Q-chain tile width (96)
T_MAX = 1000.0
KAPPA = 400.0      # 1/SIGMA_INIT^2
EPS = 1e-5
TWO_PI = 6.283185307179586
# ln(1/(2*pi*i0e(400)))  [i0e(400) = 0.019953356281939987]
LNC = 2.076480848703078
# cos(sqrt(u)) on u in [0, pi^2] (|delta| folded to [0,pi]), minimax power
# basis c0..c5 (max err 1.75e-6 -> 7e-4 on log p0; tolerance is 2e-2)
COS_COEF = [0.9999982503105576, -0.4999925129381312, 0.0416590259231213,
            -0.0013857591185452258, 2.419643469550081e-05,
            -2.1969780329048054e-07]
# degree-8 Taylor-fit coefficients for the f32 fallback path (err 4e-14)
COS_COEF8 = [1.00000000e+00, -5.00000000e-01, 4.16666666e-02, -1.38888885e-03,
             2.48015646e-05, -2.75566515e-07, 2.08651966e-09, -1.13535474e-11,
             4.13131734e-14]

_COMPILED = {}

# fast-path dtype mode: "bf16" (2-byte tiles, PE 1cyc/row, DVE 2x modes) or
# "f32r" (f32 tiles, matmul operands viewed as float32r: PE 1cyc/row on wide
# matmuls at full f32 storage precision).  Chosen empirically on hardware.
FAST_MODE = "bf16"


def _taylor_deg(x, tol, lo):
    """Smallest d with x^(d+1)/(d+1)! < tol."""
    d = lo
    term = x ** (d + 1) / math.factorial(d + 1)
    while term > tol and d < 40:
        d += 1
        term *= x / (d + 1)
    return d


def _plan(anorm):
    """Choose (k_bits, deg_p, deg_r, mode) from ||A||_inf.  The time grid is
    T0 = T_MAX/2^k_bits; every squaring level applies one bit of the
    quantized delay."""
    xa = T_MAX * float(anorm)
    if xa <= 0.0:
        return 3, 4, 3, FAST_MODE

    def pick(c_lvl, c_pre, c_tay, tol_r, tol_p_num):
        k0 = max(3, min(16, math.ceil(math.log2(max(xa / 0.9, 2.0)))))
        best = None
        for k in range(max(3, k0 - 1), min(16, k0 + 2) + 1):
            x0 = xa / (1 << k)
            tol_p = min(max(tol_p_num / 2 ** (k / 2), 5e-8), 2e-5)
            dp = _taylor_deg(x0, tol_p, 4)
            dr = _taylor_deg(x0, tol_r, 3)
            cost = (k - 1) * c_lvl + (dp - 1) * c_pre + dr * c_tay
            if best is None or cost < best[0]:
                best = (cost, k, dp, dr)
        return best[1], best[2], best[3]

    fast_c = {"bf16": (2.2, 1.6, 0.6), "f32r": (2.5, 1.7, 0.7)}[FAST_MODE]
    k, dp, dr = pick(*fast_c, 2e-4, 3e-4)
    if k <= 10:
        return k, dp, dr, FAST_MODE
    k, dp, dr = pick(7.6, 4.5, 0.7, 1e-6, 3e-5)
    return k, dp, dr, "f32"


def _build_bass(k_bits, deg_p, deg_r, mode):
    """Construct the Bass program (SPMD; identical on all 8 cores)."""
    import concourse.tile as tile
    from concourse import bacc, mybir

    F32 = mybir.dt.float32
    R32 = mybir.dt.float32r
    BF = mybir.dt.bfloat16 if mode == "bf16" else F32
    MDT = BF if mode == "bf16" else mybir.dt.uint8   # mask dtype
    AF = mybir.ActivationFunctionType
    OP = mybir.AluOpType
    cos_coef = COS_COEF if mode != "f32" else COS_COEF8

    def mv(ap):
        # matmul operand view: f32r reinterpretation in f32r mode
        return ap.bitcast(R32) if mode == "f32r" else ap

    nc = bacc.Bacc("TRN2", target_bir_lowering=False, debug=False)

    def din(name, shape, dt=F32):
        return nc.dram_tensor(name, shape, dt, kind="ExternalInput").ap()

    d_xx = din("xx", [P, 5 * N + 3 * P], BF)   # packed X / X^T pieces
    d_cmir = din("cmir", [P, NCH + BL])        # [c_mesh chunks | init rep]
    d_qtab = din("qtab", [P, (deg_r + 1) * QW], BF)  # [rdk | one-hot]
    d_msk = din("msk", [P, k_bits * QW], MDT)  # bit masks (0/1), x3 chunks
    d_out = nc.dram_tensor("terms", [1, BL], F32, kind="ExternalOutput").ap()

    with tile.TileContext(nc) as tc:
        with (
            tc.tile_pool(name="const", bufs=1) as cpool,
            tc.tile_pool(name="mats", bufs=4) as mpool,
            tc.tile_pool(name="qp", bufs=3) as qpool,
            tc.tile_pool(name="tp", bufs=4) as tpool,
            tc.tile_pool(name="psb", bufs=3, space="PSUM") as psb,
            tc.tile_pool(name="pst", bufs=3, space="PSUM") as pstp,
            tc.tile_pool(name="pss", bufs=2, space="PSUM") as pss,
        ):
            # ---- input DMAs: few, packed, in consumption order ------------
            # xx layout: [XT00 | XNr0 | XTr1 | XNr1 | XTr2 | XNr2 | XT0rest]
            # so each DMA lands exactly what the next prelude matmuls need.
            XXW = 5 * N + 3 * P
            cuts = [0, P + N, P + 3 * N, P + 5 * N, XXW]
            XXT = []
            for j, (a, b) in enumerate(zip(cuts[:-1], cuts[1:])):
                xx_j = cpool.tile([P, b - a], BF, tag=f"xx{j}")
                XXT.append(xx_j)
            CMIR = cpool.tile([P, NCH + BL], F32, tag="cmir")
            QTAB = cpool.tile([P, (deg_r + 1) * QW], BF, tag="qtab")
            MSK = cpool.tile([P, k_bits * QW], MDT, tag="msk")
            # alternate the two HWDGE queues so issue overlaps (each
            # dma_start costs ~650ns of its sequencer)
            engs = [nc.sync, nc.scalar, nc.sync, nc.scalar]
            for j, (a, b) in enumerate(zip(cuts[:-1], cuts[1:])):
                engs[j].dma_start(XXT[j][:], d_xx[:, a:b])
            nc.sync.dma_start(QTAB[:], d_qtab[:])
            nc.scalar.dma_start(CMIR[:], d_cmir[:])
            nc.sync.dma_start(MSK[:], d_msk[:])
            CM = CMIR[:, 0:NCH]
            IREP = CMIR[:, NCH:NCH + BL]
            RDK = QTAB[:, 0:deg_r * QW]
            OH = QTAB[:, deg_r * QW:(deg_r + 1) * QW]

            def _xx(off, w):
                for j, (a, b) in enumerate(zip(cuts[:-1], cuts[1:])):
                    if a <= off and off + w <= b:
                        return XXT[j][:, off - a:off - a + w]
                raise AssertionError("xx slice crosses DMA boundary")

            def xn_s(c):
                # XN row-chunk c (rhs of prelude matmuls), contiguous
                return _xx(P + 2 * c * N, N)

            def xt_s(c, i):
                # XT block (row-chunk c, piece i) for prelude/taylor lhsT
                if c == 0:
                    o = 0 if i == 0 else P + 5 * N + (i - 1) * P
                else:
                    o = P + (2 * c - 1) * N + i * P
                return _xx(o, P)

            ONES = cpool.tile([P, 1], BF, tag="ones")
            nc.vector.memset(ONES[:], 1.0)
            BEXP = cpool.tile([P, 1], F32, tag="bexp")
            nc.vector.memset(BEXP[:], LNC - KAPPA)
            BLN0 = cpool.tile([1, 1], F32, tag="bln0")
            nc.vector.memset(BLN0[:], 0.0)
            LDUM = cpool.tile([1, 1], F32, tag="ldum")
            # identities built on device (no DMA)
            EYE = cpool.tile([P, NCH * N], BF, tag="eye")
            nc.vector.memset(EYE[:], 1.0)
            nc.gpsimd.affine_select(
                EYE[:].rearrange("p (c n) -> p c n", c=NCH),
                EYE[:].rearrange("p (c n) -> p c n", c=NCH),
                pattern=[[-P, NCH], [1, N]], compare_op=OP.is_equal,
                fill=0.0, base=0, channel_multiplier=-1,
            )
            E120 = cpool.tile([P, P], BF, tag="e120")
            nc.vector.memset(E120[:], 1.0)
            nc.gpsimd.affine_select(
                E120[:], E120[:], pattern=[[1, P]], compare_op=OP.is_equal,
                fill=0.0, base=0, channel_multiplier=-1,
            )

            def mm_group(ps, lhsT_of, rhs_of, i):
                # lhsT_of(c, i) -> [P,P] slice; rhs_of(c) -> [P,w] slice
                for c in range(NCH):
                    nc.tensor.matmul(
                        ps[:], lhsT=mv(lhsT_of(c, i)), rhs=mv(rhs_of(c)),
                        start=(c == 0), stop=(c == NCH - 1),
                    )

            def tile_b(tile_, c, i):
                return tile_[:, c * N + i * P: c * N + i * P + P]

            # ---- p0 (von Mises) in Q-layout [P, 3*BL] ---------------------
            Q0 = qpool.tile([P, QW], BF, tag="q")
            deg_c = len(cos_coef) - 1
            for c in range(NCH):
                qs = Q0[:, c * BL:(c + 1) * BL]
                dl = tpool.tile([P, BL], F32, tag="t0")
                # delta = init - c_mesh  (cos is even, sign irrelevant)
                nc.vector.tensor_scalar(dl[:], IREP[:], CM[:, c:c + 1], None,
                                        op0=OP.subtract)
                ab = tpool.tile([P, BL], F32, tag="t1")
                nc.scalar.activation(ab[:], dl[:], AF.Abs)
                fl = tpool.tile([P, BL], F32, tag="t2")
                nc.vector.tensor_scalar(fl[:], ab[:], -1.0, TWO_PI,
                                        op0=OP.mult, op1=OP.add)
                w = tpool.tile([P, BL], F32, tag="t3")
                nc.vector.tensor_tensor(w[:], ab[:], fl[:], op=OP.min)
                u = tpool.tile([P, BL], F32, tag="t0")
                nc.vector.tensor_tensor(u[:], w[:], w[:], op=OP.mult)
                h = tpool.tile([P, BL], F32, tag="t1")
                nc.vector.tensor_scalar(h[:], u[:], cos_coef[deg_c],
                                        cos_coef[deg_c - 1],
                                        op0=OP.mult, op1=OP.add)
                heng = nc.gpsimd if c == 1 else nc.vector
                for k in range(deg_c - 2, -1, -1):
                    heng.tensor_tensor(h[:], h[:], u[:], op=OP.mult)
                    heng.tensor_scalar(h[:], h[:], cos_coef[k], None,
                                       op0=OP.add)
                # p0 = exp(kappa*cos - kappa + lnC)
                nc.scalar.activation(qs, h[:], AF.Exp, bias=BEXP[:],
                                     scale=KAPPA)

            # ---- Q-chain step emitters (off the M-chain critical path) ----
            # residual Taylor: V <- Q0 + rdk_k*(X V), k=deg_r..1
            taylor_state = {"V": Q0, "k": deg_r}

            def taylor_step():
                k = taylor_state["k"]
                if k < 1:
                    return
                Vc = taylor_state["V"]
                ps = pss.tile([P, QW], F32, tag="ap")
                for i in range(NCH):
                    for c in range(NCH):
                        nc.tensor.matmul(
                            ps[:, i * BL:(i + 1) * BL],
                            lhsT=mv(xt_s(c, i)),
                            rhs=mv(Vc[:, c * BL:(c + 1) * BL]),
                            start=(c == 0), stop=(c == NCH - 1),
                        )
                Vn = qpool.tile([P, QW], BF, tag="v")
                nc.vector.tensor_tensor(Vn[:], ps[:],
                                        RDK[:, (k - 1) * QW: k * QW],
                                        op=OP.mult)
                nc.gpsimd.tensor_tensor(Vn[:], Vn[:], Q0[:], op=OP.add)
                taylor_state["V"] = Vn
                taylor_state["k"] = k - 1

            def apply_bit(lhsT_of, q_tile, bit, blend_src=None):
                # Qn = bit ? M_j @ q : blend_src   (9 narrow mms + one blend)
                # the pass-through copy goes first: it only needs q, so it
                # overlaps the matmuls instead of serializing after them
                Qn = qpool.tile([P, QW], BF, tag="q")
                nc.gpsimd.tensor_copy(
                    Qn[:], (q_tile if blend_src is None else blend_src)[:])
                ps = pss.tile([P, QW], F32, tag="ap")
                for i in range(NCH):
                    for c in range(NCH):
                        nc.tensor.matmul(
                            ps[:, i * BL:(i + 1) * BL],
                            lhsT=mv(lhsT_of(c, i)),
                            rhs=mv(q_tile[:, c * BL:(c + 1) * BL]),
                            start=(c == 0), stop=(c == NCH - 1),
                        )
                nc.vector.copy_predicated(
                    Qn[:], MSK[:, bit * QW:(bit + 1) * QW], ps[:])
                return Qn

            # ---- prelude: ascending Taylor S = I + sum X^k/k! -------------
            # S and T live as three per-row-chunk [P,N] tiles so every
            # evacuation is a whole-tile write: the dependency tracker is
            # tile/bank-granular, and single-writer tiles keep consumers
            # from waiting on unrelated evacuations.
            Srows = []
            for c in range(NCH):
                s_c = mpool.tile([P, N], BF, tag=f"S{c}")
                nc.vector.tensor_tensor(s_c[:], xn_s(c),
                                        EYE[:, c * N:(c + 1) * N], op=OP.add)
                Srows.append(s_c)
            Trows = None
            for k in range(2, deg_p + 1):
                Tn = []
                for i in range(NCH):
                    t_i = mpool.tile([P, N], BF, tag=f"T{i}")
                    Tn.append(t_i)
                if Trows is None:
                    rhs_of = xn_s
                else:
                    rhs_of = (lambda c, _T=Trows: _T[c][:])
                for i in range(NCH):
                    ps = psb.tile([P, N], F32, tag="sq")
                    mm_group(ps, xt_s, rhs_of, i)
                    # scale-evac feeds the next step's matmuls; the S
                    # accumulation runs on DVE off the PE path
                    if i == 1:
                        nc.gpsimd.tensor_scalar(Tn[i][:], ps[:], 1.0 / k,
                                                None, op0=OP.mult)
                    else:
                        nc.scalar.mul(Tn[i][:], ps[:], 1.0 / k)
                    nc.vector.tensor_tensor(Srows[i][:], Srows[i][:],
                                            Tn[i][:], op=OP.add)
                Trows = Tn
                taylor_step()
            while taylor_state["k"] >= 1:   # drain (off critical path)
                taylor_step()
            QB = taylor_state["V"]

            # preload the Ln table set mid-kernel: reading the last p0
            # exp's output pins this AFTER the exps in the ACT schedule, so
            # the natural_log set switch lands in chain idle time instead of
            # on the loss critical path
            nc.scalar.activation(LDUM[:], Q0[0:1, QW - 1:QW], AF.Ln,
                                 bias=BLN0[:], scale=1.0)

            # MT lives as three PIECE-major tiles: MTp[i] holds piece i of
            # all three MT rows, i.e. exactly the stationary set the next
            # level's matmul group i consumes -- one trio of transposes
            # fills one PSUM bank, one [P,N] copy fills one tile.
            def transpose_trio(MTpn, Sr, ib, eng):
                pt = pstp.tile([P, N], BF, tag="tr")
                for cp in range(NCH):
                    nc.tensor.transpose(
                        mv(pt[:, cp * P:(cp + 1) * P]),
                        mv(Sr[ib][:, cp * P:(cp + 1) * P]),
                        mv(E120[:]),
                    )
                if eng is nc.vector:
                    nc.vector.tensor_copy(MTpn[ib][:], pt[:])
                elif eng is nc.scalar:
                    nc.scalar.copy(MTpn[ib][:], pt[:])
                else:
                    nc.gpsimd.tensor_copy(MTpn[ib][:], pt[:])

            def new_mtp():
                out = []
                for i in range(NCH):
                    mtp_i = mpool.tile([P, N], BF, tag=f"MTp{i}")
                    out.append(mtp_i)
                return out

            def mtp_acc(MTp):
                return lambda c, i: MTp[i][:, c * P:(c + 1) * P]

            tr_engs = (nc.vector, nc.scalar, nc.vector)
            mtps = [new_mtp()]
            for ib in range(NCH):
                transpose_trio(mtps[0], Srows, ib, tr_engs[ib])

            # ---- chain: squarings, with commuting bit applies lagged two
            # levels so the Q-chain (p0 -> residual Taylor) has time to
            # finish off the critical path.  All M_j share eigenvectors, so
            # bit applies can run in any order.
            n_sq = k_bits - 2
            for s in range(1, n_sq + 1):
                Sn = []
                for i in range(NCH):
                    sn_i = mpool.tile([P, N], BF, tag=f"S{i}")
                    Sn.append(sn_i)
                for i in range(NCH):
                    ps = psb.tile([P, N], F32, tag="sq")
                    mm_group(ps, mtp_acc(mtps[s - 1]),
                             lambda c, _S=Srows: _S[c][:], i)
                    if i == 0:
                        nc.gpsimd.tensor_copy(Sn[i][:], ps[:])
                    elif i == 1:
                        nc.scalar.copy(Sn[i][:], ps[:])
                    else:
                        # the last row gates this level's transposes: split
                        # it across DVE+ACT so it lands fastest
                        h = N // 2
                        nc.vector.tensor_copy(Sn[i][:, 0:h], ps[:, 0:h])
                        nc.scalar.copy(Sn[i][:, h:N], ps[:, h:N])
                MTpn = new_mtp()
                transpose_trio(MTpn, Sn, 0, tr_engs[0])
                transpose_trio(MTpn, Sn, 1, tr_engs[1])
                if s >= 2:
                    QB = apply_bit(mtp_acc(mtps[s - 2]), QB, s - 2)
                transpose_trio(MTpn, Sn, 2, tr_engs[2])
                mtps.append(MTpn)
                Srows = Sn

            # ---- remaining bits: k-3, then single + double apply of M_{k-2}
            if k_bits >= 3:
                QB = apply_bit(mtp_acc(mtps[n_sq - 1]), QB, k_bits - 3)
            top = mtp_acc(mtps[n_sq])
            QB = apply_bit(top, QB, k_bits - 2)
            psy = pss.tile([P, QW], F32, tag="ap")
            for i in range(NCH):
                for c in range(NCH):
                    nc.tensor.matmul(
                        psy[:, i * BL:(i + 1) * BL],
                        lhsT=mv(top(c, i)),
                        rhs=mv(QB[:, c * BL:(c + 1) * BL]),
                        start=(c == 0), stop=(c == NCH - 1),
                    )
            Y1 = qpool.tile([P, QW], BF, tag="v")
            nc.vector.tensor_copy(Y1[:], psy[:])
            Vf = apply_bit(top, Y1, k_bits - 1, blend_src=QB)

            # ---- selection + loss terms -----------------------------------
            sel = psb.tile([1, BL], F32, tag="sq")
            tmp = tpool.tile([P, QW], BF, tag="sel")
            nc.vector.tensor_tensor(tmp[:], Vf[:], OH[:], op=OP.mult)
            for c in range(NCH):
                nc.tensor.matmul(sel[:], lhsT=mv(ONES[:]),
                                 rhs=mv(tmp[:, c * BL:(c + 1) * BL]),
                                 start=(c == 0), stop=(c == NCH - 1))
            # ln(relu(psel)+eps) via exponent/mantissa split: the HW Ln
            # table degrades for huge args (psel can reach ~1e20 in the
            # weak-diffusion regime), so compute ln(m) + e*ln2 with m in
            # [1,2), which keeps the table in its accurate range.
            I32 = mybir.dt.int32
            rl = tpool.tile([1, BL], F32, tag="r0")
            nc.vector.tensor_scalar(rl[:], sel[:], 0.0, EPS,
                                    op0=OP.max, op1=OP.add)
            xi = rl[:].bitcast(I32)
            et = tpool.tile([1, BL], I32, tag="r2")
            nc.vector.tensor_scalar(et[:], xi, 23, None,
                                    op0=OP.arith_shift_right)
            ef = tpool.tile([1, BL], F32, tag="r3")
            nc.vector.tensor_copy(ef[:], et[:])
            mi = tpool.tile([1, BL], I32, tag="r4")
            nc.vector.tensor_scalar(mi[:], xi, 0x007FFFFF, 0x3F800000,
                                    op0=OP.bitwise_and, op1=OP.bitwise_or)
            lnm = tpool.tile([1, BL], F32, tag="r5")
            nc.scalar.activation(lnm[:], mi[:].bitcast(F32), AF.Ln,
                                 bias=BLN0[:], scale=1.0)
            terms = tpool.tile([1, BL], F32, tag="r1")
            # ef holds the biased exponent; fold the -127*ln2 into the mult
            nc.vector.tensor_scalar(terms[:], ef[:], 0.6931471805599453,
                                    -88.02969193111305,
                                    op0=OP.mult, op1=OP.add)
            nc.vector.tensor_tensor(terms[:], terms[:], lnm[:], op=OP.add)
            nc.sync.dma_start(d_out[:], terms[:])

    nc.compile()
    return nc


def _host_prep(c_mesh, gtheta, sigma_diff, init_color, delay_t, report_color):
    """Host-side glue: operator assembly (replicating reference f32 ops),
    plan selection, and per-core index/bit/layout arrays."""
    import ml_dtypes
    f32 = np.float32
    c = np.asarray(c_mesh, dtype=f32)
    g = np.asarray(gtheta, dtype=f32)
    s = np.asarray(sigma_diff, dtype=f32)[0]
    init = np.asarray(init_color, dtype=f32)
    t = np.asarray(delay_t, dtype=f32)
    rep = np.asarray(report_color, dtype=f32)

    d = (c[1] - c[0]).astype(f32)
    eye = np.eye(N, dtype=f32)
    up = np.roll(eye, -1, axis=1)
    dn = np.roll(eye, 1, axis=1)
    D1 = ((up - dn) / (f32(2.0) * d)).astype(f32)
    D2 = ((up - f32(2.0) * eye + dn) / (d * d)).astype(f32)
    A = ((s ** f32(2.0)) / f32(2.0) * D2 - D1 * g[None, :]).astype(f32)

    anorm = np.abs(A.astype(np.float64)).sum(axis=1).max()
    k_bits, deg_p, deg_r, mode = plan = _plan(anorm)
    bf = ml_dtypes.bfloat16 if mode == "bf16" else f32
    mdt = ml_dtypes.bfloat16 if mode == "bf16" else np.uint8
    T0 = T_MAX / (1 << k_bits)
    X = (A * f32(T0)).astype(f32)

    m = np.floor(t.astype(np.float64) / T0).astype(np.int64)
    m = np.clip(m, 0, (1 << k_bits) - 1)
    r = (t.astype(np.float64) - m * T0) / T0  # in X = T0*A units
    bits = ((m[:, None] >> np.arange(k_bits)[None, :]) & 1)     # [B, K]
    idx = np.argmin(np.abs(c[None, :] - rep[:, None]), axis=1)

    # packed matrix buffer in DMA/consumption order:
    # [XT00 | XNr0 | XTr1 | XNr1 | XTr2 | XNr2 | XT0rest]
    XT_ = np.ascontiguousarray(X.T)
    xx = np.concatenate([
        XT_[0:P, 0:P],
        X[0:P, :], XT_[P:2 * P, :],
        X[P:2 * P, :], XT_[2 * P:3 * P, :],
        X[2 * P:3 * P, :], XT_[0:P, P:N],
    ], axis=1)
    cm = np.ascontiguousarray(c.reshape(NCH, P).T)
    shared = {"xx": xx.astype(bf)}
    in_maps = []
    for core in range(NCORES):
        sl = slice(core * BL, (core + 1) * BL)
        irep = np.broadcast_to(init[sl][None, :], (P, BL)).astype(f32)
        cmir = np.concatenate([cm, irep], axis=1).astype(f32)
        # bit j replicated x3 (one copy per Q chunk) at [j*QW:(j+1)*QW]
        msk = np.broadcast_to(
            np.tile(bits[sl].T.reshape(k_bits, 1, BL), (1, NCH, 1))
            .reshape(1, k_bits * QW), (P, k_bits * QW)).astype(mdt)
        rdk = np.empty((deg_r, BL), f32)
        for k in range(1, deg_r + 1):
            rdk[k - 1] = (r[sl] / k).astype(f32)
        rdk = np.tile(rdk.reshape(deg_r, 1, BL), (1, NCH, 1)).reshape(
            1, deg_r * QW)
        oh = np.zeros((NCH, P, BL), f32)
        for b, ix in enumerate(idx[sl]):
            oh[ix // P, ix % P, b] = 1.0
        oh = oh.transpose(1, 0, 2).reshape(P, QW)
        qtab = np.concatenate(
            [np.broadcast_to(rdk, (P, deg_r * QW)), oh], axis=1).astype(bf)
        in_maps.append(dict(shared, cmir=cmir, msk=np.ascontiguousarray(msk),
                            qtab=np.ascontiguousarray(qtab)))
    return plan, in_maps


def _get_nc(plan):
    if plan not in _COMPILED:
        _COMPILED[plan] = _build_bass(*plan)
    return _COMPILED[plan]


def kernel(**inputs):
    from concourse.bass_utils import run_bass_kernel_spmd

    plan, in_maps = _host_prep(
        inputs["c_mesh"], inputs["gtheta"], inputs["sigma_diff"],
        inputs["init_color"], inputs["delay_t"], inputs["report_color"],
    )
    nc = _get_nc(plan)
    res = run_bass_kernel_spmd(nc, in_maps, list(range(NCORES)))
    terms = np.concatenate(
        [np.asarray(res.results[k]["terms"]).reshape(-1) for k in range(NCORES)]
    )
    loss = -np.mean(terms.astype(np.float64))
    return np.asarray(loss, dtype=np.float32)
